# revision 1
# baseline (speedup 1.0000x reference)
"""Trainium2 Bass kernel for nn_CrossModalAttention (B=4, Sq=Sk=2048, D=512, H=8).

Self-contained 8-core SPMD program: core c handles batch c//2, query-half c%2
(SQ=1024 of 2048 queries). Cores fully independent, no collectives.

Schedule (v2): the ACT engine's exp over the score matrix (H*SQ*SK elements,
128 instrs of [128,1024]) is the hard floor (~133us busy). Everything else is
arranged so ACT runs saturated from ~5us on:
  - K/V/Q projections are split into fine-grained PSUM chains (K per
    (dc,512-key window), V per (128-key chunk, 4-head group), Q per (dc,
    512-query half)) emitted in deadline order BEFORE the attention loop;
    the Tile list-scheduler trickles them into PE slack between St/PV work.
  - PSUM: St double-buffered (4 banks) + per-head context accumulator
    (1 bank, 8 qs x 64 dk) + row-sum L accumulator (1 bank, via [128,1]
    matmuls against a ones column) + 2 projection-chain banks = 8.
  - C stored bf16 so the PE transposes run 1 cycle/row; transposes for head
    pair p woven in after head 2p+1 completes (reusing projection banks).
  - DVE keeps PSUM-reading work (bias adds, normalize, LN stats); the idle
    GPSIMD engine precomputes resid+bo and does half the LN finals.
  - LayerNorm final scale fused to 2 scalar_tensor_tensor ops per row-tile.
  - Input DMAs issued in parallel across SP/ACT/DVE/POOL queues.
"""
import sys
sys.path.insert(0, "/opt/trn_rl_repo")
import numpy as np

from contextlib import ExitStack

import concourse.bass as bass
import concourse.mybir as mybir
import concourse.tile as tile
from concourse import bacc
from concourse.masks import make_identity

FP32 = mybir.dt.float32
BF16 = mybir.dt.bfloat16
P = 128


def build(SQ=1024, SK=2048, D=512, H=8, num_devices=8, dbg=False):
    DK = D // H                   # 64
    DC = D // P                   # 4
    KC = SK // P                  # 16 key chunks
    NQT = SQ // P                 # 8 query subtiles
    QF = 512
    NQF = SQ // QF                # 2
    NW = SK // 512                # 4 key windows
    HG = 2                        # head groups (4 heads each) for V chains
    HPG = H // HG                 # 4
    Alu = mybir.AluOpType
    Act = mybir.ActivationFunctionType

    nc = bacc.Bacc("TRN2", target_bir_lowering=False, debug=False,
                   num_devices=num_devices)

    def din(name, shape, dt=FP32):
        return nc.dram_tensor(name, list(shape), dt, kind="ExternalInput").ap()

    qT = din("qT", (D, SQ), BF16)
    kT = din("kT", (D, SK), BF16)
    vT = din("vT", (D, SK), BF16)
    resid = din("resid", (SQ, D))
    w_dram = {n: din(n, (D, D), BF16) for n in ("wq", "wk", "wv", "wo")}
    # packed consts: [bq 0:4][bk 4:8][bv 8:520][bo 520:1032][g 1032:1544][b 1544:2056]
    cpack = din("cpack", (P, 2 * DC + 4 * D))
    out = nc.dram_tensor("out", [SQ, D], FP32, kind="ExternalOutput").ap()

    with tile.TileContext(nc) as tc, ExitStack() as ctx:
        consts = ctx.enter_context(tc.tile_pool(name="consts", bufs=1))
        acts = ctx.enter_context(tc.tile_pool(name="acts", bufs=1))
        pj_ps = ctx.enter_context(tc.tile_pool(name="pj_ps", bufs=2, space="PSUM"))
        st_ps = ctx.enter_context(tc.tile_pool(name="st_ps", bufs=2, space="PSUM"))
        c_ps = ctx.enter_context(tc.tile_pool(name="c_ps", bufs=1, space="PSUM"))
        pt_pool = ctx.enter_context(tc.tile_pool(name="pt", bufs=8))
        lr_pool = ctx.enter_context(tc.tile_pool(name="lr", bufs=3))
        ep = ctx.enter_context(tc.tile_pool(name="ep", bufs=3))
        ot_pool = ctx.enter_context(tc.tile_pool(name="ot", bufs=2))

        # ---------------- input DMAs, spread across engine queues ----------------
        qT_sb = acts.tile([P, DC, SQ], BF16, tag="qTin")
        qTr = qT.rearrange("(c p) q -> p c q", p=P)
        nc.sync.dma_start(qT_sb[:, :, 0:QF], qTr[:, :, 0:QF])
        nc.scalar.dma_start(qT_sb[:, :, QF:SQ], qTr[:, :, QF:SQ])

        wsb = {}
        for n in ("wq", "wk", "wv", "wo"):
            wsb[n] = acts.tile([P, DC, D], BF16, tag=n, name=n)
        nc.sync.dma_start(wsb["wq"][:], w_dram["wq"].rearrange("(c p) o -> p c o", p=P))

        kin = acts.tile([P, DC, SK], BF16, tag="kin")
        kTr = kT.rearrange("(c p) s -> p c s", p=P)
        nc.scalar.dma_start(kin[:, :, 0:512], kTr[:, :, 0:512])
        nc.scalar.dma_start(wsb["wk"][:], w_dram["wk"].rearrange("(c p) o -> p c o", p=P))

        vin = acts.tile([P, DC, SK], BF16, tag="vin")
        vTr = vT.rearrange("(c p) s -> p c s", p=P)
        nc.gpsimd.dma_start(vin[:, :, 0:512], vTr[:, :, 0:512])
        nc.sync.dma_start(wsb["wv"][:], w_dram["wv"].rearrange("(c p) o -> p c o", p=P))

        # consts: only bq/bk/bv needed during projections; bo/g/b loaded late
        cp_sb = consts.tile([P, 2 * DC + 4 * D], FP32, tag="cpack")
        nc.scalar.dma_start(cp_sb[:, 0:2 * DC + D], cpack[:, 0:2 * DC + D])

        bq_sb = cp_sb[:, 0:DC]
        bk_sb = cp_sb[:, DC:2 * DC]
        bv_sb = cp_sb[:, 2 * DC:2 * DC + D]
        bo_sb = cp_sb[:, 2 * DC + D:2 * DC + 2 * D]
        g_sb = cp_sb[:, 2 * DC + 2 * D:2 * DC + 3 * D]
        b_sb = cp_sb[:, 2 * DC + 3 * D:2 * DC + 4 * D]

        ident = consts.tile([P, P], BF16, tag="ident")
        make_identity(nc, ident[:])
        ones_bf = consts.tile([P, 1], BF16, tag="ones")
        nc.vector.memset(ones_bf[:], 1.0)
        eps_sb = consts.tile([P, 1], FP32, tag="eps")
        nc.vector.memset(eps_sb[:], 1e-5)

        # ---------------- resident activations ----------------
        Qt = acts.tile([P, DC, SQ], BF16, tag="Qt")
        Kt = acts.tile([P, DC, SK], BF16, tag="Kt")
        V = acts.tile([P, KC, H, DK + 1], BF16, tag="V")
        nc.vector.memset(V[:, :, :, DK], 1.0)
        C = acts.tile([P, NQT, D], BF16, tag="C")
        Ct = acts.tile([P, DC, SQ], BF16, tag="Ct")
        t0_all = acts.tile([P, NQT, D], FP32, tag="t0")
        rs2_all = acts.tile([P, NQT, D], FP32, tag="rs2")
        mv_all = acts.tile([P, NQT, 2], FP32, tag="mv")
        sdev = acts.tile([P, NQT], FP32, tag="sdev")
        rstd_all = acts.tile([P, NQT], FP32, tag="rstd")

        # ---------------- projection chains ----------------
        def q_chain(dc, qf):
            ps = pj_ps.tile([P, D], FP32, tag="pj", name=f"q{dc}_{qf}")[:, 0:QF]
            for i in range(DC):
                nc.tensor.matmul(ps, lhsT=wsb["wq"][:, i, dc * P:(dc + 1) * P],
                                 rhs=qT_sb[:, i, qf * QF:(qf + 1) * QF],
                                 start=(i == 0), stop=(i == DC - 1))
            nc.vector.tensor_scalar_add(Qt[:, dc, qf * QF:(qf + 1) * QF], ps,
                                        bq_sb[:, dc:dc + 1])

        def k_chain(dc, w):
            ps = pj_ps.tile([P, D], FP32, tag="pj", name=f"k{dc}_{w}")[:, 0:512]
            for i in range(DC):
                nc.tensor.matmul(ps, lhsT=wsb["wk"][:, i, dc * P:(dc + 1) * P],
                                 rhs=kin[:, i, w * 512:(w + 1) * 512],
                                 start=(i == 0), stop=(i == DC - 1))
            nc.vector.tensor_scalar_add(Kt[:, dc, w * 512:(w + 1) * 512], ps,
                                        bk_sb[:, dc:dc + 1])

        def v_chain(sc, hg):
            cw = hg * HPG * DK                      # 256-col offset
            ps = pj_ps.tile([P, D], FP32, tag="pj",
                            name=f"v{sc}_{hg}")[:, 0:HPG * DK]
            for i in range(DC):
                nc.tensor.matmul(ps, lhsT=vin[:, i, sc * P:(sc + 1) * P],
                                 rhs=wsb["wv"][:, i, cw:cw + HPG * DK],
                                 start=(i == 0), stop=(i == DC - 1))
            nc.vector.tensor_tensor(
                V[:, sc, hg * HPG:(hg + 1) * HPG, 0:DK],
                ps.rearrange("p (h d) -> p h d", d=DK),
                bv_sb[:, cw:cw + HPG * DK].rearrange("p (h d) -> p h d", d=DK),
                Alu.add)

        # prologue: minimum to start (h0, kc0)
        q_chain(0, 0)
        q_chain(0, 1)
        k_chain(0, 0)
        for sc in range(4):
            v_chain(sc, 0)
        # background, deadline order (scheduler trickles into PE slack);
        # later kin/vin windows DMA'd just ahead of their first consumer
        nc.scalar.dma_start(kin[:, :, 512:1024], kTr[:, :, 512:1024])
        nc.gpsimd.dma_start(vin[:, :, 512:1024], vTr[:, :, 512:1024])
        k_chain(0, 1)
        for sc in range(4, 8):
            v_chain(sc, 0)
        nc.scalar.dma_start(kin[:, :, 1024:1536], kTr[:, :, 1024:1536])
        nc.gpsimd.dma_start(vin[:, :, 1024:1536], vTr[:, :, 1024:1536])
        k_chain(0, 2)
        for sc in range(8, 12):
            v_chain(sc, 0)
        nc.scalar.dma_start(kin[:, :, 1536:2048], kTr[:, :, 1536:2048])
        nc.gpsimd.dma_start(vin[:, :, 1536:2048], vTr[:, :, 1536:2048])
        k_chain(0, 3)
        for sc in range(12, 16):
            v_chain(sc, 0)
        q_chain(1, 0)
        q_chain(1, 1)
        for w in range(NW):
            k_chain(1, w)
        for sc in range(16):
            v_chain(sc, 1)
        q_chain(2, 0)
        q_chain(2, 1)
        for w in range(NW):
            k_chain(2, w)
        q_chain(3, 0)
        q_chain(3, 1)
        for w in range(NW):
            k_chain(3, w)

        # ---------------- attention ----------------
        JB = 4                      # qs per cps bank (65 f32 each)
        NQB = NQT // JB

        def emit_transpose(dc):
            for qs in range(NQT):
                tp = pj_ps.tile([P, P], BF16, tag="pj", name=f"tp{dc}_{qs}")
                nc.tensor.transpose(tp[:], C[:, qs, dc * P:(dc + 1) * P], ident[:])
                nc.vector.tensor_copy(out=Ct[:, dc, qs * P:(qs + 1) * P], in_=tp[:])

        for h in range(H):
            dc_h = h // 2
            off = (h % 2) * DK
            cps = c_ps.tile([P, NQB, 512], FP32, tag="cps", name=f"cps{h}")
            for kc in range(KC):
                st = st_ps.tile([P, SQ], FP32, tag="st")
                for qf in range(NQF):
                    nc.tensor.matmul(
                        st[:, qf * QF:(qf + 1) * QF],
                        lhsT=Kt[off:off + DK, dc_h, kc * P:(kc + 1) * P],
                        rhs=Qt[off:off + DK, dc_h, qf * QF:(qf + 1) * QF],
                        start=True, stop=True)
                pt = pt_pool.tile([P, SQ], BF16, tag="pt")
                nc.scalar.activation(pt[:], st[:], Act.Exp, scale=0.125)
                for qs in range(NQT):
                    jcol = (qs % JB) * (DK + 1)
                    nc.tensor.matmul(cps[:, qs // JB, jcol:jcol + DK + 1],
                                     lhsT=pt[:, qs * P:(qs + 1) * P],
                                     rhs=V[:, kc, h, :],
                                     start=(kc == 0 and qs % JB == 0),
                                     stop=(kc == KC - 1 and qs % JB == JB - 1))
            cview = cps[:, :, 0:JB * (DK + 1)].rearrange(
                "p b (j x) -> p b j x", x=DK + 1)
            lr = lr_pool.tile([P, NQB, JB, 1], FP32, tag="lr")
            nc.vector.reciprocal(lr[:], cview[:, :, :, DK:DK + 1])
            nc.vector.tensor_tensor(
                C[:, :, h * DK:(h + 1) * DK].rearrange(
                    "p (b j) d -> p b j d", j=JB),
                cview[:, :, :, 0:DK],
                lr[:].to_broadcast((P, NQB, JB, DK)),
                Alu.mult)
            if h == 5:
                # prefetch epilogue inputs while attention still runs
                nc.gpsimd.dma_start(cp_sb[:, 2 * DC + D:],
                                    cpack[:, 2 * DC + D:])
                nc.gpsimd.dma_start(wsb["wo"][:],
                                    w_dram["wo"].rearrange("(c p) o -> p c o", p=P))
                for qs in range(NQT):
                    rs = ep.tile([P, D], FP32, tag="rs")
                    nc.sync.dma_start(rs[:], resid[qs * P:(qs + 1) * P, :])
                    nc.gpsimd.tensor_tensor(rs2_all[:, qs, :], rs[:], bo_sb,
                                            Alu.add)
            if h % 2 == 1:
                emit_transpose(h // 2)

        # ---------------- out-proj + layernorm ----------------
        def ln_final(qs):
            y = ep.tile([P, D], FP32, tag="y")
            nc.vector.scalar_tensor_tensor(
                y[:], t0_all[:, qs, :], mv_all[:, qs, 0:1],
                g_sb, Alu.subtract, Alu.mult)
            ot = ot_pool.tile([P, D], FP32, tag="ot")
            nc.vector.scalar_tensor_tensor(
                ot[:], y[:], rstd_all[:, qs:qs + 1],
                b_sb, Alu.mult, Alu.add)
            nc.sync.dma_start(out[qs * P:(qs + 1) * P, :], ot[:])

        HQ = NQT // 2
        for qs in range(NQT):
            ps = st_ps.tile([P, SQ], FP32, tag="st", name=f"ops{qs}")[:, 0:D]
            for dc in range(DC):
                nc.tensor.matmul(ps, lhsT=Ct[:, dc, qs * P:(qs + 1) * P],
                                 rhs=wsb["wo"][:, dc, :],
                                 start=(dc == 0), stop=(dc == DC - 1))
            t0 = t0_all[:, qs, :]
            nc.vector.tensor_tensor(t0, ps, rs2_all[:, qs, :], Alu.add)
            st6 = ep.tile([P, 6], FP32, tag="st6")
            nc.vector.bn_stats(st6[:], t0)
            nc.vector.bn_aggr(mv_all[:, qs, :], st6[:])
            if qs == HQ - 1:
                # finalize first half while PE runs the remaining chains
                nc.scalar.activation(sdev[:, 0:HQ], mv_all[:, 0:HQ, 1],
                                     Act.Sqrt, bias=eps_sb[:])
                nc.vector.reciprocal(rstd_all[:, 0:HQ], sdev[:, 0:HQ])
                for q2 in range(HQ):
                    ln_final(q2)

        nc.scalar.activation(sdev[:, HQ:NQT], mv_all[:, HQ:NQT, 1],
                             Act.Sqrt, bias=eps_sb[:])
        nc.vector.reciprocal(rstd_all[:, HQ:NQT], sdev[:, HQ:NQT])
        for qs in range(HQ, NQT):
            ln_final(qs)

        if dbg:
            for nm, t in (("dbg_C", C), ("dbg_Ct", Ct)):
                dt_ = nc.dram_tensor(nm, list(t[:].shape), BF16,
                                     kind="ExternalOutput").ap()
                nc.sync.dma_start(dt_, t[:])
            for nm, t in (("dbg_t0", t0_all), ("dbg_rs2", rs2_all),
                          ("dbg_mv", mv_all), ("dbg_rstd", rstd_all)):
                dt_ = nc.dram_tensor(nm, list(t[:].shape), FP32,
                                     kind="ExternalOutput").ap()
                nc.sync.dma_start(dt_, t[:])

    nc.compile()
    return nc


def make_in_map(query_slice, key_b, value_b, wq, bq, wk, bk, wv, bv, wo, bo,
                ln_g, ln_b):
    """Host-side shard prep for one core. query_slice [SQ, D]; key_b/value_b [SK, D]."""
    import ml_dtypes
    D = wq.shape[0]
    DC = D // P
    f = np.float32
    bf = ml_dtypes.bfloat16

    def rep(v):
        return np.broadcast_to(v.astype(f), (P, D))

    def plat(v):
        return v.astype(f).reshape(DC, P).T

    cpack = np.concatenate(
        [plat(bq), plat(bk), rep(bv), rep(bo), rep(ln_g), rep(ln_b)], axis=1)
    return {
        "qT": np.ascontiguousarray(query_slice.T.astype(f).astype(bf)),
        "kT": np.ascontiguousarray(key_b.T.astype(f).astype(bf)),
        "vT": np.ascontiguousarray(value_b.T.astype(f).astype(bf)),
        "resid": np.ascontiguousarray(query_slice.astype(f)),
        "wq": np.ascontiguousarray(wq.astype(f).astype(bf)),
        "wk": np.ascontiguousarray(wk.astype(f).astype(bf)),
        "wv": np.ascontiguousarray(wv.astype(f).astype(bf)),
        "wo": np.ascontiguousarray(wo.astype(f).astype(bf)),
        "cpack": np.ascontiguousarray(cpack),
    }


_NC_CACHE = {}


def _get_nc():
    if "nc" not in _NC_CACHE:
        _NC_CACHE["nc"] = build(SQ=1024, SK=2048, D=512, H=8, num_devices=8)
    return _NC_CACHE["nc"]


def kernel(query, key, value, wq, bq, wk, bk, wv, bv, wo, bo, ln_g, ln_b):
    from concourse.bass_utils import run_bass_kernel_spmd
    query = np.asarray(query, dtype=np.float32)
    key = np.asarray(key, dtype=np.float32)
    value = np.asarray(value, dtype=np.float32)
    B, SQ_FULL, D = query.shape
    SQH = SQ_FULL // 2
    nc = _get_nc()
    in_maps = []
    for c in range(8):
        b, qh = c // 2, c % 2
        in_maps.append(make_in_map(
            query[b, qh * SQH:(qh + 1) * SQH, :], key[b], value[b],
            np.asarray(wq), np.asarray(bq), np.asarray(wk), np.asarray(bk),
            np.asarray(wv), np.asarray(bv), np.asarray(wo), np.asarray(bo),
            np.asarray(ln_g), np.asarray(ln_b)))
    res = run_bass_kernel_spmd(nc, in_maps, core_ids=list(range(8)))
    out = np.empty((B, SQ_FULL, D), np.float32)
    for c, r in enumerate(res.results):
        out[c // 2, (c % 2) * SQH:((c % 2) + 1) * SQH, :] = r["out"]
    return out



# revision 6
# speedup vs baseline: 1.1788x; 1.1788x over previous
"""Trainium2 Bass kernel for nn_CrossModalAttention (B=4, Sq=Sk=2048, D=512, H=8).

Self-contained 8-core SPMD program: core c handles batch c//2, query-half c%2
(SQ=1024 of 2048 queries). Cores fully independent, no collectives.

v3 design (vs v2 baseline at 216.7us):
  - QKV projections in fp8e4 DoubleRow (2 contraction rows/cycle): weights
    host-scaled x8 (fp8 normal range), the 1/8 factors folded into the exp
    scale and the epilogue residual add. PE proj cost drops 4x.
  - P*V in fp8 DoubleRow: exp output pt stored as key-pairs [128,2,1024],
    V as [128,kcp,2,H,65] (ones column accumulates the softmax denominator).
  - Softmax exp split across TWO engines: ACT runs exact Exp (output fp8e4,
    globally shifted by e^-2 to stay under fp8e4's 240 max - softmax
    invariant), DVE runs a 1-instruction fast-exp: bits = round(s*c1 + c2)
    saturated to [0,255] and reinterpreted as fp8e4 (linear-mantissa approx,
    ~2.7% rel err on those chunks only). This splits the 133us exp wall.
  - Scores stay bf16 (PE has slack; keeps precision); C/Ct bf16.
  - Epilogue: residual+bias precomputed on the idle Pool engine; LN stats/
    finals on DVE; half finalized early.
"""
import sys
sys.path.insert(0, "/opt/trn_rl_repo")
import numpy as np

from contextlib import ExitStack
from functools import partial

import concourse.bass as bass
import concourse.mybir as mybir
import concourse.tile as tile
from concourse import bacc
from concourse.masks import make_identity

FP32 = mybir.dt.float32
BF16 = mybir.dt.bfloat16
FP8 = mybir.dt.float8e4
U8 = mybir.dt.uint8
P = 128
LOG2E8 = 8.0 / float(np.log(2.0))  # 11.5416: fp8e4 bits per unit of ln-space
EXP_SHIFT = 3.25                   # global exp shift (softmax-invariant);
                                   # keeps exp(s_max=8.3 - shift) < fp8e4 max 240


def build(SQ=1024, SK=2048, D=512, H=8, num_devices=8, dbg=False):
    DK = D // H                   # 64
    DC = D // P                   # 4
    KC = SK // P                  # 16 key chunks
    KCP = KC // 2                 # 8 key-chunk pairs
    NQT = SQ // P                 # 8 query subtiles
    QF = 512
    NQF = SQ // QF                # 2
    NW = SK // 512                # 4 key windows
    JB = 4                        # qs per cps bank
    Alu = mybir.AluOpType
    Act = mybir.ActivationFunctionType
    DRM = mybir.MatmulPerfMode.DoubleRow

    K_EXP = 1.0 / 512.0           # exp scale: 1/(64 carried *8*8) / sqrt(64)
    C1 = K_EXP * LOG2E8
    C2 = 56.0 - EXP_SHIFT * LOG2E8 - 0.345
    # fast-exp (DVE) tiles per head, placed on odd kc of evenly spread pairs
    DVE_EXP = [4, 5, 5, 6, 6, 6, 6, 6]

    nc = bacc.Bacc("TRN2", target_bir_lowering=False, debug=False,
                   num_devices=num_devices)

    def din(name, shape, dt=FP32):
        return nc.dram_tensor(name, list(shape), dt, kind="ExternalInput").ap()

    qT = din("qT", (P, 2, 2, SQ), FP8)
    kT = din("kT", (P, 2, 2, SK), FP8)
    vT = din("vT", (P, 2, 2, SK), FP8)
    resid = din("resid", (SQ, D))
    wq_d = din("wq", (P, 2, 2, D), FP8)
    wk_d = din("wk", (P, 2, 2, D), FP8)
    wv_d = din("wv", (P, 2, 2, D), FP8)
    wo_d = din("wo", (P, DC, D), BF16)
    # packed consts: [bq8 0:4][bk8 4:8][bv8 8:520][bo 520:1032][g][b]
    cpack = din("cpack", (P, 2 * DC + 4 * D))
    out = nc.dram_tensor("out", [SQ, D], FP32, kind="ExternalOutput").ap()

    with tile.TileContext(nc) as tc, ExitStack() as ctx:
        consts = ctx.enter_context(tc.tile_pool(name="consts", bufs=1))
        acts = ctx.enter_context(tc.tile_pool(name="acts", bufs=1))
        pj_ps = ctx.enter_context(tc.tile_pool(name="pj_ps", bufs=2, space="PSUM"))
        st_ps = ctx.enter_context(tc.tile_pool(name="st_ps", bufs=2, space="PSUM"))
        c_ps = ctx.enter_context(tc.tile_pool(name="c_ps", bufs=1, space="PSUM"))
        pt_pool = ctx.enter_context(tc.tile_pool(name="pt", bufs=3))
        lr_pool = ctx.enter_context(tc.tile_pool(name="lr", bufs=3))
        ep = ctx.enter_context(tc.tile_pool(name="ep", bufs=3))
        ot_pool = ctx.enter_context(tc.tile_pool(name="ot", bufs=2))

        # ---------------- input DMAs, spread across engine queues -----------
        wq_sb = acts.tile([P, 2, 2, D], FP8, tag="wq")
        nc.sync.dma_start(wq_sb[:], wq_d)
        qT_sb = acts.tile([P, 2, 2, SQ], FP8, tag="qTin")
        nc.sync.dma_start(qT_sb[:], qT)
        wk_sb = acts.tile([P, 2, 2, D], FP8, tag="wk")
        nc.scalar.dma_start(wk_sb[:], wk_d)
        kin = acts.tile([P, 2, 2, SK], FP8, tag="kin")
        nc.scalar.dma_start(kin[:, :, :, 0:SK // 2], kT[:, :, :, 0:SK // 2])
        nc.sync.dma_start(kin[:, :, :, SK // 2:SK], kT[:, :, :, SK // 2:SK])
        wv_sb = acts.tile([P, 2, 2, D], FP8, tag="wv")
        nc.gpsimd.dma_start(wv_sb[:], wv_d)
        vin = acts.tile([P, 2, 2, SK], FP8, tag="vin")
        nc.gpsimd.dma_start(vin[:, :, :, 0:SK // 2], vT[:, :, :, 0:SK // 2])
        nc.gpsimd.dma_start(vin[:, :, :, SK // 2:SK], vT[:, :, :, SK // 2:SK])

        # consts: only bq/bk/bv needed during projections; bo/g/b loaded late
        cp_sb = consts.tile([P, 2 * DC + 4 * D], FP32, tag="cpack")
        nc.scalar.dma_start(cp_sb[:, 0:2 * DC + D], cpack[:, 0:2 * DC + D])

        bq_sb = cp_sb[:, 0:DC]
        bk_sb = cp_sb[:, DC:2 * DC]
        bv_sb = cp_sb[:, 2 * DC:2 * DC + D]
        bo_sb = cp_sb[:, 2 * DC + D:2 * DC + 2 * D]
        g_sb = cp_sb[:, 2 * DC + 2 * D:2 * DC + 3 * D]
        b_sb = cp_sb[:, 2 * DC + 3 * D:2 * DC + 4 * D]

        ident = consts.tile([P, P], BF16, tag="ident")
        make_identity(nc, ident[:])
        eps_sb = consts.tile([P, 1], FP32, tag="eps")
        nc.vector.memset(eps_sb[:], 1e-5)
        nbias = consts.tile([P, 1], FP32, tag="nbias")
        nc.vector.memset(nbias[:], -EXP_SHIFT)

        # ---------------- resident activations ----------------
        Qt = acts.tile([P, DC, SQ], BF16, tag="Qt")
        Kt = acts.tile([P, DC, SK], BF16, tag="Kt")
        V = acts.tile([P, KCP, 2, H, DK + 1], FP8, tag="V")
        nc.vector.memset(V[:, :, :, :, DK], 1.0)
        C = acts.tile([P, NQT, D], BF16, tag="C")
        Ct = acts.tile([P, DC, SQ], BF16, tag="Ct")
        t0_all = acts.tile([P, NQT, D], FP32, tag="t0")
        rs2_all = acts.tile([P, NQT, D], FP32, tag="rs2")
        mv_all = acts.tile([P, NQT, 2], FP32, tag="mv")
        sdev = acts.tile([P, NQT], FP32, tag="sdev")
        rstd_all = acts.tile([P, NQT], FP32, tag="rstd")

        # ---------------- fp8 DoubleRow projection chains ----------------
        def q_chain(dc, qf):
            ps = pj_ps.tile([P, QF], FP32, tag="pj", name=f"q{dc}_{qf}")
            for i in range(2):
                nc.tensor.matmul(ps[:], lhsT=wq_sb[:, i, :, dc * P:(dc + 1) * P],
                                 rhs=qT_sb[:, i, :, qf * QF:(qf + 1) * QF],
                                 start=(i == 0), stop=(i == 1), perf_mode=DRM)
            nc.vector.tensor_scalar_add(Qt[:, dc, qf * QF:(qf + 1) * QF], ps[:],
                                        bq_sb[:, dc:dc + 1])

        def k_chain(dc, w):
            ps = pj_ps.tile([P, QF], FP32, tag="pj", name=f"k{dc}_{w}")
            for i in range(2):
                nc.tensor.matmul(ps[:], lhsT=wk_sb[:, i, :, dc * P:(dc + 1) * P],
                                 rhs=kin[:, i, :, w * 512:(w + 1) * 512],
                                 start=(i == 0), stop=(i == 1), perf_mode=DRM)
            nc.vector.tensor_scalar_add(Kt[:, dc, w * 512:(w + 1) * 512], ps[:],
                                        bk_sb[:, dc:dc + 1])

        def v_chain(sc):
            ps = pj_ps.tile([P, QF], FP32, tag="pj", name=f"v{sc}")
            for i in range(2):
                nc.tensor.matmul(ps[:], lhsT=vin[:, i, :, sc * P:(sc + 1) * P],
                                 rhs=wv_sb[:, i, :, :],
                                 start=(i == 0), stop=(i == 1), perf_mode=DRM)
            nc.vector.tensor_tensor(
                V[:, sc // 2, sc % 2, :, 0:DK],
                ps[:].rearrange("p (h d) -> p h d", d=DK),
                bv_sb.rearrange("p (h d) -> p h d", d=DK),
                Alu.add)

        # prologue: minimum to start h0
        q_chain(0, 0)
        q_chain(0, 1)
        k_chain(0, 0)
        v_chain(0)
        v_chain(1)

        # remaining chains injected into PE slack during attention
        inject = {}

        def add_inj(h, kcp, fn, *args):
            inject.setdefault((h, kcp), []).append(partial(fn, *args))

        add_inj(0, 0, k_chain, 0, 1)
        add_inj(0, 0, v_chain, 2)
        add_inj(0, 0, v_chain, 3)
        add_inj(0, 1, k_chain, 0, 2)
        add_inj(0, 1, v_chain, 4)
        add_inj(0, 1, v_chain, 5)
        add_inj(0, 2, k_chain, 0, 3)
        add_inj(0, 2, v_chain, 6)
        add_inj(0, 2, v_chain, 7)
        add_inj(0, 3, v_chain, 8)
        add_inj(0, 3, v_chain, 9)
        add_inj(0, 4, v_chain, 10)
        add_inj(0, 4, v_chain, 11)
        add_inj(0, 5, v_chain, 12)
        add_inj(0, 5, v_chain, 13)
        add_inj(0, 6, v_chain, 14)
        add_inj(0, 6, v_chain, 15)
        add_inj(0, 7, q_chain, 1, 0)
        add_inj(0, 7, q_chain, 1, 1)
        for w in range(NW):
            add_inj(1, w, k_chain, 1, w)
        add_inj(1, 6, q_chain, 2, 0)
        add_inj(1, 7, q_chain, 2, 1)
        for w in range(NW):
            add_inj(2, w, k_chain, 2, w)
        add_inj(3, 6, q_chain, 3, 0)
        add_inj(3, 7, q_chain, 3, 1)
        for w in range(NW):
            add_inj(4, w, k_chain, 3, w)

        # ---------------- attention ----------------
        def emit_transpose(dc):
            for qs in range(NQT):
                tp = pj_ps.tile([P, P], BF16, tag="pj", name=f"tp{dc}_{qs}")
                nc.tensor.transpose(tp[:], C[:, qs, dc * P:(dc + 1) * P], ident[:])
                nc.vector.tensor_copy(out=Ct[:, dc, qs * P:(qs + 1) * P], in_=tp[:])

        for h in range(H):
            dc_h = h // 2
            off = (h % 2) * DK
            ndve = DVE_EXP[h]
            dve_pairs = {int(i * KCP / ndve) for i in range(ndve)}
            cps = c_ps.tile([P, 2, 512], FP32, tag="cps", name=f"cps{h}")
            for kcp in range(KCP):
                pt = pt_pool.tile([P, 2, SQ], FP8, tag="pt")
                for jj in range(2):
                    kc = 2 * kcp + jj
                    st = st_ps.tile([P, SQ], FP32, tag="st")
                    for qf in range(NQF):
                        nc.tensor.matmul(
                            st[:, qf * QF:(qf + 1) * QF],
                            lhsT=Kt[off:off + DK, dc_h, kc * P:(kc + 1) * P],
                            rhs=Qt[off:off + DK, dc_h, qf * QF:(qf + 1) * QF],
                            start=True, stop=True)
                    if jj == 1 and kcp in dve_pairs:
                        # fast-exp on DVE: affine to fp8e4 bit space, u8
                        # convert saturates [0,255] and rounds to nearest
                        nc.vector.tensor_scalar(
                            pt[:, jj, :].bitcast(U8), st[:], C1, C2,
                            Alu.mult, Alu.add)
                    else:
                        nc.scalar.activation(pt[:, jj, :], st[:], Act.Exp,
                                             scale=K_EXP, bias=nbias[:])
                for qs in range(NQT):
                    jcol = (qs % JB) * (DK + 1)
                    nc.tensor.matmul(cps[:, qs // JB, jcol:jcol + DK + 1],
                                     lhsT=pt[:, :, qs * P:(qs + 1) * P],
                                     rhs=V[:, kcp, :, h, :],
                                     start=(kcp == 0 and qs % JB == 0),
                                     stop=(kcp == KCP - 1 and qs % JB == JB - 1),
                                     perf_mode=DRM)
                for fn in inject.get((h, kcp), []):
                    fn()
            cview = cps[:, :, 0:JB * (DK + 1)].rearrange(
                "p b (j x) -> p b j x", x=DK + 1)
            lr = lr_pool.tile([P, 2, JB, 1], FP32, tag="lr")
            nc.vector.reciprocal(lr[:], cview[:, :, :, DK:DK + 1])
            nc.vector.tensor_tensor(
                C[:, :, h * DK:(h + 1) * DK].rearrange(
                    "p (b j) d -> p b j d", j=JB),
                cview[:, :, :, 0:DK],
                lr[:].to_broadcast((P, 2, JB, DK)),
                Alu.mult)
            if h == 5:
                # prefetch epilogue inputs while attention still runs
                nc.gpsimd.dma_start(cp_sb[:, 2 * DC + D:],
                                    cpack[:, 2 * DC + D:])
                wo_sb = acts.tile([P, DC, D], BF16, tag="wo")
                nc.gpsimd.dma_start(wo_sb[:], wo_d)
                for qs in range(NQT):
                    rs = ep.tile([P, D], FP32, tag="rs")
                    nc.sync.dma_start(rs[:], resid[qs * P:(qs + 1) * P, :])
                    nc.gpsimd.tensor_tensor(rs2_all[:, qs, :], rs[:], bo_sb,
                                            Alu.add)
            if h % 2 == 1:
                emit_transpose(h // 2)

        # ---------------- out-proj + layernorm ----------------
        def ln_final(qs):
            y = ep.tile([P, D], FP32, tag="y")
            nc.vector.scalar_tensor_tensor(
                y[:], t0_all[:, qs, :], mv_all[:, qs, 0:1],
                g_sb, Alu.subtract, Alu.mult)
            ot = ot_pool.tile([P, D], FP32, tag="ot")
            nc.vector.scalar_tensor_tensor(
                ot[:], y[:], rstd_all[:, qs:qs + 1],
                b_sb, Alu.mult, Alu.add)
            nc.sync.dma_start(out[qs * P:(qs + 1) * P, :], ot[:])

        HQ = NQT // 2
        for qs in range(NQT):
            ps = pj_ps.tile([P, D], FP32, tag="pj", name=f"ops{qs}")
            for dc in range(DC):
                nc.tensor.matmul(ps[:], lhsT=Ct[:, dc, qs * P:(qs + 1) * P],
                                 rhs=wo_sb[:, dc, :],
                                 start=(dc == 0), stop=(dc == DC - 1))
            nc.vector.scalar_tensor_tensor(
                t0_all[:, qs, :], ps[:], 0.125, rs2_all[:, qs, :],
                Alu.mult, Alu.add)
            st6 = ep.tile([P, 6], FP32, tag="st6")
            nc.vector.bn_stats(st6[:], t0_all[:, qs, :])
            nc.vector.bn_aggr(mv_all[:, qs, :], st6[:])
            if qs == HQ - 1:
                # finalize first half while PE runs the remaining chains
                nc.scalar.activation(sdev[:, 0:HQ], mv_all[:, 0:HQ, 1],
                                     Act.Sqrt, bias=eps_sb[:])
                nc.vector.reciprocal(rstd_all[:, 0:HQ], sdev[:, 0:HQ])
                for q2 in range(HQ):
                    ln_final(q2)

        nc.scalar.activation(sdev[:, HQ:NQT], mv_all[:, HQ:NQT, 1],
                             Act.Sqrt, bias=eps_sb[:])
        nc.vector.reciprocal(rstd_all[:, HQ:NQT], sdev[:, HQ:NQT])
        for qs in range(HQ, NQT):
            ln_final(qs)

        if dbg:
            for nm, t, dt_ in (("dbg_Qt", Qt, BF16), ("dbg_Kt", Kt, BF16),
                               ("dbg_V", V, FP8), ("dbg_C", C, BF16),
                               ("dbg_Ct", Ct, BF16), ("dbg_t0", t0_all, FP32),
                               ("dbg_rs2", rs2_all, FP32),
                               ("dbg_mv", mv_all, FP32)):
                d = nc.dram_tensor(nm, list(t[:].shape), dt_,
                                   kind="ExternalOutput").ap()
                nc.sync.dma_start(d, t[:])

    nc.compile()
    return nc


def make_in_map(query_slice, key_b, value_b, wq, bq, wk, bk, wv, bv, wo, bo,
                ln_g, ln_b):
    """Host-side shard prep for one core. query_slice [SQ, D]; key_b/value_b [SK, D]."""
    import ml_dtypes
    D = wq.shape[0]
    DC = D // P
    f = np.float32
    bf = ml_dtypes.bfloat16
    f8 = ml_dtypes.float8_e4m3

    def rep(v):
        return np.broadcast_to(v.astype(f), (P, D))

    def plat(v):
        return v.astype(f).reshape(DC, P).T

    def xdr(x):
        # x [S, D] -> fp8 [128, 2, 2, S]; fin = i*256 + j*128 + k
        xT = np.ascontiguousarray(x.astype(f).T)
        return np.ascontiguousarray(
            xT.reshape(2, 2, P, -1).transpose(2, 0, 1, 3).astype(f8))

    def wdr(w):
        # w [D, D] -> x8-scaled fp8 [128, 2, 2, D]
        return np.ascontiguousarray(
            (np.asarray(w).astype(f) * 8.0).reshape(2, 2, P, D)
            .transpose(2, 0, 1, 3).astype(f8))

    cpack = np.concatenate(
        [plat(bq * 8.0), plat(bk * 8.0), rep(bv * 8.0), rep(bo), rep(ln_g),
         rep(ln_b)], axis=1)
    return {
        "qT": xdr(query_slice),
        "kT": xdr(key_b),
        "vT": xdr(value_b),
        "resid": np.ascontiguousarray(query_slice.astype(f)),
        "wq": wdr(wq),
        "wk": wdr(wk),
        "wv": wdr(wv),
        "wo": np.ascontiguousarray(
            np.asarray(wo).astype(f).reshape(DC, P, D).transpose(1, 0, 2)
            .astype(bf)),
        "cpack": np.ascontiguousarray(cpack.astype(f)),
    }


_NC_CACHE = {}


def _get_nc():
    if "nc" not in _NC_CACHE:
        _NC_CACHE["nc"] = build(SQ=1024, SK=2048, D=512, H=8, num_devices=8)
    return _NC_CACHE["nc"]


def kernel(query, key, value, wq, bq, wk, bk, wv, bv, wo, bo, ln_g, ln_b):
    from concourse.bass_utils import run_bass_kernel_spmd
    query = np.asarray(query, dtype=np.float32)
    key = np.asarray(key, dtype=np.float32)
    value = np.asarray(value, dtype=np.float32)
    B, SQ_FULL, D = query.shape
    SQH = SQ_FULL // 2
    nc = _get_nc()
    in_maps = []
    for c in range(8):
        b, qh = c // 2, c % 2
        in_maps.append(make_in_map(
            query[b, qh * SQH:(qh + 1) * SQH, :], key[b], value[b],
            np.asarray(wq), np.asarray(bq), np.asarray(wk), np.asarray(bk),
            np.asarray(wv), np.asarray(bv), np.asarray(wo), np.asarray(bo),
            np.asarray(ln_g), np.asarray(ln_b)))
    res = run_bass_kernel_spmd(nc, in_maps, core_ids=list(range(8)))
    out = np.empty((B, SQ_FULL, D), np.float32)
    for c, r in enumerate(res.results):
        out[c // 2, (c % 2) * SQH:((c % 2) + 1) * SQH, :] = r["out"]
    return out


# revision 45
# speedup vs baseline: 1.4873x; 1.2617x over previous
"""Trainium2 Bass kernel for nn_CrossModalAttention (B=4, Sq=Sk=2048, D=512, H=8).

Self-contained 8-core SPMD program: core c handles batch c//2, query-half c%2
(SQ=1024 of 2048 queries). Cores fully independent, no collectives.

v3 design (vs v2 baseline at 216.7us):
  - QKV projections in fp8e4 DoubleRow (2 contraction rows/cycle): weights
    host-scaled x8 (fp8 normal range), the 1/8 factors folded into the exp
    scale and the epilogue residual add. PE proj cost drops 4x.
  - P*V in fp8 DoubleRow: exp output pt stored as key-pairs [128,2,1024],
    V as [128,kcp,2,H,65] (ones column accumulates the softmax denominator).
  - Softmax exp split across TWO engines: ACT runs exact Exp (output fp8e4,
    globally shifted by e^-2 to stay under fp8e4's 240 max - softmax
    invariant), DVE runs a 1-instruction fast-exp: bits = round(s*c1 + c2)
    saturated to [0,255] and reinterpreted as fp8e4 (linear-mantissa approx,
    ~2.7% rel err on those chunks only). This splits the 133us exp wall.
  - Scores stay bf16 (PE has slack; keeps precision); C/Ct bf16.
  - Epilogue: residual+bias precomputed on the idle Pool engine; LN stats/
    finals on DVE; half finalized early.
"""
import sys
sys.path.insert(0, "/opt/trn_rl_repo")
import numpy as np

from contextlib import ExitStack
from functools import partial

import concourse.bass as bass
import concourse.mybir as mybir
import concourse.tile as tile
from concourse import bacc
from concourse.masks import make_identity

FP32 = mybir.dt.float32
BF16 = mybir.dt.bfloat16
FP8 = mybir.dt.float8e4
U8 = mybir.dt.uint8
P = 128
LOG2E8 = 8.0 / float(np.log(2.0))  # 11.5416: fp8e4 bits per unit of ln-space
EXP_SHIFT = 3.25                   # global exp shift (softmax-invariant);
                                   # keeps exp(s_max=8.3 - shift) < fp8e4 max 240


def build(SQ=1024, SK=2048, D=512, H=8, num_devices=8, dbg=False):
    DK = D // H                   # 64
    DC = D // P                   # 4
    KC = SK // P                  # 16 key chunks
    KCP = KC // 2                 # 8 key-chunk pairs
    NQT = SQ // P                 # 8 query subtiles
    QF = 512
    NQF = SQ // QF                # 2
    NW = SK // 512                # 4 key windows
    JB = 4                        # qs per cps bank
    Alu = mybir.AluOpType
    Act = mybir.ActivationFunctionType
    DRM = mybir.MatmulPerfMode.DoubleRow

    K_EXP = 1.0 / 512.0           # exp scale: 1/(64 carried *8*8) / sqrt(64)
    C1 = K_EXP * LOG2E8
    C2 = 56.0 - EXP_SHIFT * LOG2E8 - 0.345
    # scores are computed in two 512-query halves into two independent PSUM
    # pools (two 2-deep pipelines; a PSUM tile is only ever read by ONE
    # engine - two engines reading one tile get falsely serialized by the
    # dep tracker). Per head, ACT_N halves (of 32) run exact exp on ACT,
    # the rest fast-exp on DVE; (A,A) kcs are spread mid-head.
    ACT_N = [20, 17, 17, 17, 16, 18, 18, 18]

    nc = bacc.Bacc("TRN2", target_bir_lowering=False, debug=False,
                   num_devices=num_devices)

    def din(name, shape, dt=FP32):
        return nc.dram_tensor(name, list(shape), dt, kind="ExternalInput").ap()

    qT = din("qT", (P, 2, 2, SQ), FP8)
    kT = din("kT", (P, 2, 2, SK), FP8)
    vT = din("vT", (P, 2, 2, SK), FP8)
    resid = din("resid", (SQ, D))
    wq_d = din("wq", (P, 2, 2, D), FP8)
    wk_d = din("wk", (P, 2, 2, D), FP8)
    wv_d = din("wv", (P, 2, 2, D), FP8)
    wo_d = din("wo", (P, DC, D), BF16)
    # packed consts: [bq8 0:4][bk8 4:8][bv8 8:520][bo 520:1032][g][b], bf16
    cpack = din("cpack", (P, 2 * DC + 4 * D), BF16)
    out = nc.dram_tensor("out", [SQ, D], FP32, kind="ExternalOutput").ap()

    with tile.TileContext(nc) as tc, ExitStack() as ctx:
        consts = ctx.enter_context(tc.tile_pool(name="consts", bufs=1))
        acts = ctx.enter_context(tc.tile_pool(name="acts", bufs=1))
        pj_ps = ctx.enter_context(tc.tile_pool(name="pj_ps", bufs=2, space="PSUM"))
        stA_ps = ctx.enter_context(tc.tile_pool(name="stA_ps", bufs=2, space="PSUM"))
        stD_ps = ctx.enter_context(tc.tile_pool(name="stD_ps", bufs=2, space="PSUM"))
        c_ps = ctx.enter_context(tc.tile_pool(name="c_ps", bufs=1, space="PSUM"))
        pt_pool = ctx.enter_context(tc.tile_pool(name="pt", bufs=3))
        ptd_pool = ctx.enter_context(tc.tile_pool(name="ptd", bufs=3))
        lr_pool = ctx.enter_context(tc.tile_pool(name="lr", bufs=3))
        ep = ctx.enter_context(tc.tile_pool(name="ep", bufs=3))
        ot_pool = ctx.enter_context(tc.tile_pool(name="ot", bufs=2))

        # ---------------- input DMAs ----------------
        # All on HWDGE queues (SWDGE/Pool DMAs cost >1us of Pool engine
        # time each), ordered by first use: q chains gate everything.
        qT_sb = acts.tile([P, 2, 2, SQ], FP8, tag="qTin")
        wq_sb = acts.tile([P, 2, 2, D], FP8, tag="wq")
        kin = acts.tile([P, 2, 2, SK], FP8, tag="kin")
        vin = acts.tile([P, 2, 2, SK], FP8, tag="vin")
        wk_sb = acts.tile([P, 2, 2, D], FP8, tag="wk")
        wv_sb = acts.tile([P, 2, 2, D], FP8, tag="wv")
        cp_sb = consts.tile([P, 2 * DC + 4 * D], BF16, tag="cpack")
        # few LARGE transfers (each costs ~625ns serial HWDGE time), in
        # first-use order; epilogue inputs (wo/resid) last
        nc.sync.dma_start(wq_sb[:], wq_d)
        nc.scalar.dma_start(wk_sb[:], wk_d)
        nc.sync.dma_start(qT_sb[:], qT)
        nc.scalar.dma_start(cp_sb[:], cpack)
        nc.sync.dma_start(kin[:, :, :, 0:SK // 2], kT[:, :, :, 0:SK // 2])
        nc.scalar.dma_start(wv_sb[:], wv_d)
        nc.sync.dma_start(vin[:, :, :, 0:SK // 2], vT[:, :, :, 0:SK // 2])
        nc.scalar.dma_start(kin[:, :, :, SK // 2:SK], kT[:, :, :, SK // 2:SK])
        nc.sync.dma_start(vin[:, :, :, SK // 2:SK], vT[:, :, :, SK // 2:SK])
        wo_sb = acts.tile([P, DC, D], BF16, tag="wo")
        nc.scalar.dma_start(wo_sb[:], wo_d)
        rs_all = acts.tile([P, NQT, D], FP32, tag="rs")
        nc.sync.dma_start(rs_all[:], resid.rearrange("(q p) d -> p q d", p=P))

        bq_sb = cp_sb[:, 0:DC]
        bk_sb = cp_sb[:, DC:2 * DC]
        bv_sb = cp_sb[:, 2 * DC:2 * DC + D]
        bo_sb = cp_sb[:, 2 * DC + D:2 * DC + 2 * D]
        g_sb = cp_sb[:, 2 * DC + 2 * D:2 * DC + 3 * D]
        b_sb = cp_sb[:, 2 * DC + 3 * D:2 * DC + 4 * D]

        ident = consts.tile([P, P], BF16, tag="ident")
        make_identity(nc, ident[:])
        eps_sb = consts.tile([P, 1], FP32, tag="eps")
        nc.vector.memset(eps_sb[:], 1e-5)
        nbias = consts.tile([P, 1], FP32, tag="nbias")
        nc.vector.memset(nbias[:], -EXP_SHIFT)

        # ---------------- resident activations ----------------
        Qt = acts.tile([P, DC, SQ], BF16, tag="Qt")
        Kt = acts.tile([P, DC, SK], BF16, tag="Kt")
        V = acts.tile([P, KCP, 2, H, DK + 1], FP8, tag="V")
        nc.vector.memset(V[:, :, :, :, DK], 1.0)
        C = acts.tile([P, NQT, D], BF16, tag="C")
        Ct = acts.tile([P, DC, SQ], BF16, tag="Ct")
        t0_all = acts.tile([P, NQT, D], FP32, tag="t0")
        rs2_all = acts.tile([P, NQT, D], FP32, tag="rs2")
        accS = acts.tile([P, NQT], FP32, tag="accS")
        accQ = acts.tile([P, NQT], FP32, tag="accQ")
        mean_all = acts.tile([P, NQT], FP32, tag="mean")
        var_all = acts.tile([P, NQT], FP32, tag="var")
        sdev = acts.tile([P, NQT], FP32, tag="sdev")
        rstd_all = acts.tile([P, NQT], FP32, tag="rstd")

        # ---------------- fp8 DoubleRow projection chains ----------------
        def q_chain(dc, qf):
            ps = pj_ps.tile([P, QF], FP32, tag="pj", name=f"q{dc}_{qf}")
            for i in range(2):
                nc.tensor.matmul(ps[:], lhsT=wq_sb[:, i, :, dc * P:(dc + 1) * P],
                                 rhs=qT_sb[:, i, :, qf * QF:(qf + 1) * QF],
                                 start=(i == 0), stop=(i == 1), perf_mode=DRM)
            nc.scalar.activation(Qt[:, dc, qf * QF:(qf + 1) * QF], ps[:],
                                 Act.Identity, bias=bq_sb[:, dc:dc + 1])

        def k_chain(dc, w):
            ps = pj_ps.tile([P, QF], FP32, tag="pj", name=f"k{dc}_{w}")
            for i in range(2):
                nc.tensor.matmul(ps[:], lhsT=wk_sb[:, i, :, dc * P:(dc + 1) * P],
                                 rhs=kin[:, i, :, w * 512:(w + 1) * 512],
                                 start=(i == 0), stop=(i == 1), perf_mode=DRM)
            nc.scalar.activation(Kt[:, dc, w * 512:(w + 1) * 512], ps[:],
                                 Act.Identity, bias=bk_sb[:, dc:dc + 1])

        def v_chain(sc):
            ps = pj_ps.tile([P, QF], FP32, tag="pj", name=f"v{sc}")
            for i in range(2):
                nc.tensor.matmul(ps[:], lhsT=vin[:, i, :, sc * P:(sc + 1) * P],
                                 rhs=wv_sb[:, i, :, :],
                                 start=(i == 0), stop=(i == 1), perf_mode=DRM)
            nc.vector.tensor_tensor(
                V[:, sc // 2, sc % 2, :, 0:DK],
                ps[:].rearrange("p (h d) -> p h d", d=DK),
                bv_sb.rearrange("p (h d) -> p h d", d=DK),
                Alu.add)

        # prologue: minimum to start h0
        q_chain(0, 0)
        q_chain(0, 1)
        k_chain(0, 0)
        v_chain(0)
        v_chain(1)

        # remaining chains injected into PE slack during attention
        inject = {}

        def add_inj(h, kcp, fn, *args):
            inject.setdefault((h, kcp), []).append(partial(fn, *args))

        add_inj(0, 0, v_chain, 2)
        add_inj(0, 1, k_chain, 0, 1)
        add_inj(0, 1, v_chain, 3)
        add_inj(0, 2, v_chain, 4)
        add_inj(0, 2, v_chain, 5)
        add_inj(0, 3, k_chain, 0, 2)
        add_inj(0, 3, v_chain, 6)
        add_inj(0, 3, v_chain, 7)
        add_inj(0, 4, v_chain, 8)
        add_inj(0, 4, v_chain, 9)
        add_inj(0, 4, k_chain, 0, 3)
        add_inj(0, 5, v_chain, 10)
        add_inj(0, 5, v_chain, 11)
        add_inj(0, 6, v_chain, 12)
        add_inj(0, 6, v_chain, 13)
        add_inj(0, 7, v_chain, 14)
        add_inj(0, 7, v_chain, 15)
        add_inj(0, 7, q_chain, 1, 0)
        add_inj(0, 7, q_chain, 1, 1)
        for w in range(NW):
            add_inj(1, 2 * w, k_chain, 1, w)
        add_inj(1, 5, q_chain, 2, 0)
        add_inj(1, 7, q_chain, 2, 1)
        for w in range(NW):
            add_inj(2, 2 * w, k_chain, 2, w)
        for w in range(NW):
            add_inj(3, 2 * w, k_chain, 3, w)
        add_inj(4, 2, q_chain, 3, 0)
        add_inj(4, 5, q_chain, 3, 1)

        # ---------------- attention ----------------
        # Software-pipelined over a flat (h, kcp) unit stream: PV of unit
        # i-1 is emitted AFTER the scores+exp of unit i so the in-order PE
        # queue never head-of-line blocks on exp completion.
        def emit_transpose(dc, qs):
            tp = pj_ps.tile([P, P], BF16, tag="pj", name=f"tp{dc}_{qs}")
            nc.tensor.transpose(tp[:], C[:, qs, dc * P:(dc + 1) * P], ident[:])
            nc.vector.tensor_copy(out=Ct[:, dc, qs * P:(qs + 1) * P], in_=tp[:])

        for h in range(1, H, 2):
            dc = h // 2
            dsth = h + 1 if h < H - 1 else None
            for qs in range(NQT):
                if dsth is None:
                    break
                add_inj(dsth, 2 + qs // 2, emit_transpose, dc, qs)

        cps_tiles = {}
        pt_tiles = {}

        # per-(head, kc, half) engine choice: True = ACT exact exp
        n_aa = [max(0, ACT_N[h] - 16) for h in range(H)]
        use_act = {}
        for h in range(H):
            aa_kcs = {int((i + 1) * KC / (n_aa[h] + 1)) for i in range(n_aa[h])}
            for kc in range(KC):
                use_act[(h, kc, 0)] = True
                use_act[(h, kc, 1)] = kc in aa_kcs

        def exp_half(engA, st, out_ap):
            if engA:
                nc.scalar.activation(out_ap, st[:], Act.Exp,
                                     scale=K_EXP, bias=nbias[:])
            else:
                # fast-exp on DVE: affine to fp8e4 bit space; u8 convert
                # saturates [0,255], rounds to nearest
                nc.vector.tensor_scalar(out_ap.bitcast(U8), st[:], C1, C2,
                                        Alu.mult, Alu.add)

        def emit_scores_exp(h, kcp):
            dc_h = h // 2
            off = (h % 2) * DK
            pt = pt_pool.tile([P, 2, SQ], FP8, tag="pt")
            pt_tiles[(h, kcp)] = pt
            for jj in range(2):
                kc = 2 * kcp + jj
                stA = stA_ps.tile([P, QF], FP32, tag="stA")
                nc.tensor.matmul(
                    stA[:], lhsT=Kt[off:off + DK, dc_h, kc * P:(kc + 1) * P],
                    rhs=Qt[off:off + DK, dc_h, 0:QF], start=True, stop=True)
                exp_half(use_act[(h, kc, 0)], stA, pt[:, jj, 0:QF])
                stD = stD_ps.tile([P, QF], FP32, tag="stD")
                nc.tensor.matmul(
                    stD[:], lhsT=Kt[off:off + DK, dc_h, kc * P:(kc + 1) * P],
                    rhs=Qt[off:off + DK, dc_h, QF:SQ], start=True, stop=True)
                exp_half(use_act[(h, kc, 1)], stD, pt[:, jj, QF:SQ])

        def emit_pv(h, kcp):
            if kcp == 0:
                cps_tiles[h] = c_ps.tile([P, 2, 512], FP32, tag="cps",
                                         name=f"cps{h}")
            cps = cps_tiles[h]
            pt = pt_tiles.pop((h, kcp))
            for qs in range(NQT):
                jcol = (qs % JB) * (DK + 1)
                nc.tensor.matmul(cps[:, qs // JB, jcol:jcol + DK + 1],
                                 lhsT=pt[:, :, qs * P:(qs + 1) * P],
                                 rhs=V[:, kcp, :, h, :],
                                 start=(kcp == 0 and qs % JB == 0),
                                 stop=(kcp == KCP - 1 and qs % JB == JB - 1),
                                 perf_mode=DRM)
            if kcp == KCP - 1:
                emit_norm(h)

        def emit_norm(h):
            cps = cps_tiles.pop(h)
            cview = cps[:, :, 0:JB * (DK + 1)].rearrange(
                "p b (j x) -> p b j x", x=DK + 1)
            lr = lr_pool.tile([P, 2, JB, 1], FP32, tag="lr")
            nc.vector.reciprocal(lr[:], cview[:, :, :, DK:DK + 1])
            nc.vector.tensor_tensor(
                C[:, :, h * DK:(h + 1) * DK].rearrange(
                    "p (b j) d -> p b j d", j=JB),
                cview[:, :, :, 0:DK],
                lr[:].to_broadcast((P, 2, JB, DK)),
                Alu.mult)
            if h == 5:
                # rs2 = resid + bo on the idle Pool engine
                for qs in range(NQT):
                    nc.gpsimd.tensor_tensor(rs2_all[:, qs, :],
                                            rs_all[:, qs, :], bo_sb,
                                            Alu.add)

        units = [(h, kcp) for h in range(H) for kcp in range(KCP)]
        for idx, (h, kcp) in enumerate(units):
            emit_scores_exp(h, kcp)
            if idx >= 1:
                emit_pv(*units[idx - 1])
            for fn in inject.get((h, kcp), []):
                fn()
        emit_pv(*units[-1])
        for qs in range(NQT):
            emit_transpose(DC - 1, qs)

        # ---------------- out-proj + layernorm ----------------
        def ln_final(qs):
            y = ep.tile([P, D], FP32, tag="y")
            nc.vector.scalar_tensor_tensor(
                y[:], t0_all[:, qs, :], mean_all[:, qs:qs + 1],
                g_sb, Alu.subtract, Alu.mult)
            ot = ot_pool.tile([P, D], FP32, tag="ot")
            nc.vector.scalar_tensor_tensor(
                ot[:], y[:], rstd_all[:, qs:qs + 1],
                b_sb, Alu.mult, Alu.add)
            nc.sync.dma_start(out[qs * P:(qs + 1) * P, :], ot[:])

        def finalize(q0, q1):
            # mean = S/512, var = Q/512 - mean^2; S,Q accumulated on ACT
            nc.vector.tensor_scalar_mul(mean_all[:, q0:q1], accS[:, q0:q1],
                                        1.0 / D)
            m2 = ep.tile([P, NQT], FP32, tag="m2")
            nc.vector.tensor_tensor(m2[:, q0:q1], mean_all[:, q0:q1],
                                    mean_all[:, q0:q1], Alu.mult)
            nc.vector.scalar_tensor_tensor(
                var_all[:, q0:q1], accQ[:, q0:q1], 1.0 / D, m2[:, q0:q1],
                Alu.mult, Alu.subtract)
            nc.scalar.activation(sdev[:, q0:q1], var_all[:, q0:q1],
                                 Act.Sqrt, bias=eps_sb[:])
            nc.vector.reciprocal(rstd_all[:, q0:q1], sdev[:, q0:q1])
            for q2 in range(q0, q1):
                ln_final(q2)

        for qs in range(NQT):
            ps = pj_ps.tile([P, D], FP32, tag="pj", name=f"ops{qs}")
            for dc in range(DC):
                nc.tensor.matmul(ps[:], lhsT=Ct[:, dc, qs * P:(qs + 1) * P],
                                 rhs=wo_sb[:, dc, :],
                                 start=(dc == 0), stop=(dc == DC - 1))
            nc.vector.scalar_tensor_tensor(
                t0_all[:, qs, :], ps[:], 0.125, rs2_all[:, qs, :],
                Alu.mult, Alu.add)
            # LN sums on ACT (off the DVE-bound tail): Sq and S
            sq = ep.tile([P, D], FP32, tag="sq")
            nc.scalar.activation(sq[:], t0_all[:, qs, :], Act.Square,
                                 accum_out=accQ[:, qs:qs + 1])
            s1 = ep.tile([P, D], FP32, tag="s1")
            nc.scalar.activation(s1[:], t0_all[:, qs, :], Act.Identity,
                                 accum_out=accS[:, qs:qs + 1])
            if qs % 2 == 1:
                finalize(qs - 1, qs + 1)

        if dbg:
            for nm, t, dt_ in (("dbg_Qt", Qt, BF16), ("dbg_Kt", Kt, BF16),
                               ("dbg_V", V, FP8), ("dbg_C", C, BF16),
                               ("dbg_Ct", Ct, BF16), ("dbg_t0", t0_all, FP32),
                               ("dbg_rs2", rs2_all, FP32),
                               ("dbg_mv", mv_all, FP32)):
                d = nc.dram_tensor(nm, list(t[:].shape), dt_,
                                   kind="ExternalOutput").ap()
                nc.sync.dma_start(d, t[:])

    nc.compile()
    return nc


def make_in_map(query_slice, key_b, value_b, wq, bq, wk, bk, wv, bv, wo, bo,
                ln_g, ln_b):
    """Host-side shard prep for one core. query_slice [SQ, D]; key_b/value_b [SK, D]."""
    import ml_dtypes
    D = wq.shape[0]
    DC = D // P
    f = np.float32
    bf = ml_dtypes.bfloat16
    f8 = ml_dtypes.float8_e4m3

    def rep(v):
        return np.broadcast_to(v.astype(f), (P, D))

    def plat(v):
        return v.astype(f).reshape(DC, P).T

    def xdr(x):
        # x [S, D] -> fp8 [128, 2, 2, S]; fin = i*256 + j*128 + k
        xT = np.ascontiguousarray(x.astype(f).T)
        return np.ascontiguousarray(
            xT.reshape(2, 2, P, -1).transpose(2, 0, 1, 3).astype(f8))

    def wdr(w):
        # w [D, D] -> x8-scaled fp8 [128, 2, 2, D]
        return np.ascontiguousarray(
            (np.asarray(w).astype(f) * 8.0).reshape(2, 2, P, D)
            .transpose(2, 0, 1, 3).astype(f8))

    cpack = np.concatenate(
        [plat(bq * 8.0), plat(bk * 8.0), rep(bv * 8.0), rep(bo), rep(ln_g),
         rep(ln_b)], axis=1).astype(bf)
    return {
        "qT": xdr(query_slice),
        "kT": xdr(key_b),
        "vT": xdr(value_b),
        "resid": np.ascontiguousarray(query_slice.astype(f)),
        "wq": wdr(wq),
        "wk": wdr(wk),
        "wv": wdr(wv),
        "wo": np.ascontiguousarray(
            np.asarray(wo).astype(f).reshape(DC, P, D).transpose(1, 0, 2)
            .astype(bf)),
        "cpack": np.ascontiguousarray(cpack),
    }


_NC_CACHE = {}


def _get_nc():
    if "nc" not in _NC_CACHE:
        _NC_CACHE["nc"] = build(SQ=1024, SK=2048, D=512, H=8, num_devices=8)
    return _NC_CACHE["nc"]


def kernel(query, key, value, wq, bq, wk, bk, wv, bv, wo, bo, ln_g, ln_b):
    from concourse.bass_utils import run_bass_kernel_spmd
    query = np.asarray(query, dtype=np.float32)
    key = np.asarray(key, dtype=np.float32)
    value = np.asarray(value, dtype=np.float32)
    B, SQ_FULL, D = query.shape
    SQH = SQ_FULL // 2
    nc = _get_nc()
    in_maps = []
    for c in range(8):
        b, qh = c // 2, c % 2
        in_maps.append(make_in_map(
            query[b, qh * SQH:(qh + 1) * SQH, :], key[b], value[b],
            np.asarray(wq), np.asarray(bq), np.asarray(wk), np.asarray(bk),
            np.asarray(wv), np.asarray(bv), np.asarray(wo), np.asarray(bo),
            np.asarray(ln_g), np.asarray(ln_b)))
    res = run_bass_kernel_spmd(nc, in_maps, core_ids=list(range(8)))
    out = np.empty((B, SQ_FULL, D), np.float32)
    for c, r in enumerate(res.results):
        out[c // 2, (c % 2) * SQH:((c % 2) + 1) * SQH, :] = r["out"]
    return out
